# revision 24
# baseline (speedup 1.0000x reference)
"""DeepSeek-V3.1 MoE block (B=2,S=512,H=1024,I=512,E=64,topK=8) on 8 trn2 cores.

Strategy v3 (expert-parallel, sparse dispatch, fp8 weights+activations):
  - Host: fp64 router, top-8/token, per-expert token gather with counts-sorted
    slot assignment (8 experts/core), capacity = per-slot max count.
  - Expert weights AND xg activations in fp8 e3m4 (4 mantissa bits, ~1.2% rms;
    scales 64/4 folded into the silu input scale and routing coef); routed
    output in fp8 e4m3 (scale 64 folded into coef). Shared-expert path stays
    bf16 end-to-end (it dominates output rms, its error hits ~1:1).
    Measured fro_rel 1.385e-2 vs the 2e-2 gate.
  - Device, per expert slot: one token group of C<=256 columns.
      gate/up: stationary fp8 W-chunks [128h,128i] x moving X^T [128h,C] fp8,
        accumulated over 8 h-chunks into PSUM chunks (2 banks each).
      silu(scale=1/256) on ACT; a = silu(g)*u (DVE) * coef (DVE, coef row
        pre-broadcast from host, bf16).
      down: stationary fp8 Wd-chunks [128i,128h] x moving a [128i,C] bf16 ->
        y^T [128h,C] PSUM (tokens on the free dim: no remainder-block waste),
        copied to fp8e4 and DMA'd transposed (host untransposes, /64).
  - PE software pipeline: down(e-1) issued between up(e) and gate(e+1) so
    ACT/DVE work of expert e overlaps PE down of e-1; the shared expert is
    interleaved BEFORE the last (smallest) fp8 expert so its serial bf16
    tail chain hides under that expert's compute.
  - DMA rings: sync ring carries ONLY the 1.5 MB expert slabs (SDMA engines
    round-robin across busy rings, so anything else there starves the
    knife-edge weight stream); xg/cf/was/xs ride scalar (was in six 0.5 MB
    chunks gated by marker writes so the dep-free 3 MB load cannot bunch);
    outputs ride gpsimd, tail outputs per-PSUM-group on sync.
  - DMA/core ~18.9 MB (was ~33 MB bf16): experts 12.6 fp8 + shared 3.3 bf16 +
    acts ~3 MB -> ~53 us at the ~358 GB/s HBM-per-core roofline; PE ~54-57 us
    busy (864 matmuls, avg N~140). Measured 80-84 us NEFF span (baseline 106;
    chip clock state drifts +-15% run-to-run).
"""
import os as _os, sys
try:
    import concourse  # noqa: F401  (env-provided, e.g. axon boot path)
except ImportError:
    for _p in ('/root/.axon_site/_ro/trn_rl_repo', '/opt/trn_rl_repo'):
        if _os.path.isdir(_p) and _p not in sys.path:
            sys.path.append(_p)
import numpy as np
from ml_dtypes import bfloat16, float8_e3m4, float8_e4m3

B, S, H, I, E, TOPK = 2, 512, 1024, 512, 64, 8
T = B * S
NCORES = 8
ELOC = E // NCORES
HC, IC = H // 128, I // 128
TSH = T // NCORES  # shared-expert tokens per core (128)
W = HC * I         # 4096: per-matrix partition-major width
WSCALE = 64.0      # fp8 weight scale (weights ~N(0,0.02) -> sigma ~1.28)
TGMAX = 256        # max token-group columns (PSUM chunk pair per bank)
XSCALE = 4.0       # fp8 activation scale for xg (x ~N(0,1))
YSCALE = 64.0      # fp8 output scale for ygT (folded into cf)

LAST_RESULT = None  # BassKernelResults of the most recent run (for harness)


def _pmajor(a, nchunk):
    """[nchunk*128, F] -> partition-major [128, nchunk*F] (chunk-row-major)."""
    F = a.shape[1]
    return np.ascontiguousarray(
        a.reshape(nchunk, 128, F).transpose(1, 0, 2).reshape(128, nchunk * F))


def _build(caps, reps=1):
    import concourse.bacc as bacc
    import concourse.mybir as mybir
    from concourse import tile

    F32 = mybir.dt.float32
    BF16 = mybir.dt.bfloat16
    FP8 = mybir.dt.float8e3
    FP8E4 = mybir.dt.float8e4
    SILU = mybir.ActivationFunctionType.Silu
    COPY = mybir.ActivationFunctionType.Copy

    xoff = np.concatenate([[0], np.cumsum([HC * c for c in caps])])
    yoff = np.concatenate([[0], np.cumsum(caps)])
    XW, YW = int(xoff[-1]), int(yoff[-1])
    maxcap = max(caps)

    nc = bacc.Bacc("TRN2", target_bir_lowering=False, debug=False)

    xg_d = nc.dram_tensor("xg", [128, XW], FP8, kind="ExternalInput")
    wa_d = nc.dram_tensor("wa", [ELOC, 128, 3 * W], FP8, kind="ExternalInput")
    cf_d = nc.dram_tensor("cf", [128, YW], BF16, kind="ExternalInput")
    xs_d = nc.dram_tensor("xs", [128, HC * TSH], BF16, kind="ExternalInput")
    was_d = nc.dram_tensor("was", [128, 3 * W], BF16, kind="ExternalInput")
    ygT_d = nc.dram_tensor("ygT", [128, XW], FP8E4, kind="ExternalOutput")
    ysT_d = nc.dram_tensor("ysT", [128, HC * TSH], BF16, kind="ExternalOutput")

    with tile.TileContext(nc) as tc:
        with (
            tc.tile_pool(name="const", bufs=1) as cpool,
            tc.tile_pool(name="wp", bufs=3) as wpool,
            tc.tile_pool(name="xp", bufs=3) as xpool,
            tc.tile_pool(name="ap", bufs=2) as apool,
            tc.tile_pool(name="yp", bufs=2) as ypool,
            tc.tile_pool(name="ps", bufs=1, space="PSUM") as pspool,
        ):
            cf_bc = cpool.tile([128, YW], BF16)
            was_t = cpool.tile([128, 3 * W], BF16)
            xs_t = cpool.tile([128, HC * TSH], BF16)

            # HAM pre-warm: ~16 dep-free matmuls on a zeroed tile keep the PE
            # busy through the ~6us DMA ramp, so the clock gate is at 8/8
            # (2.4 GHz) when the first real matmuls issue. Uses the gate PSUM
            # tag before its first real use; results are never read.
            warm = cpool.tile([128, 512], BF16)
            nc.vector.memzero(warm[:])
            warm_ps = pspool.tile([128, 1024], F32, tag="g", bufs=1)
            for _ in range(12):
                nc.tensor.matmul(warm_ps[:, :512], warm[:, :128], warm[:],
                                 start=True, stop=True)

            def emit_gu(xg_t, C_slot, tg, cols, wa_t, silu_scale, cf_c0):
                """Gate+up+silu+mul for one token group; returns the a tile."""
                g_ps = pspool.tile([128, 1024], F32, tag="g", bufs=1)
                u_ps = pspool.tile([128, 1024], F32, tag="u", bufs=1)
                for base, ps in ((0, g_ps), (W, u_ps)):
                    for t in range(IC):
                        off = (t // 2) * 512 + (t % 2) * cols
                        for h in range(HC):
                            nc.tensor.matmul(
                                ps[:, off:off + cols],
                                wa_t[:, base + h * I + t * 128:
                                     base + h * I + (t + 1) * 128],
                                xg_t[:, h * C_slot + tg:h * C_slot + tg + cols],
                                start=(h == 0), stop=(h == HC - 1))
                s_sb = apool.tile([128, 1024], F32, tag="s")
                a1 = apool.tile([128, 1024], BF16, tag="a1")
                for o in (0, 512):
                    nc.scalar.activation(s_sb[:, o:o + 2 * cols],
                                         g_ps[:, o:o + 2 * cols], SILU,
                                         scale=silu_scale)
                    nc.vector.tensor_mul(a1[:, o:o + 2 * cols],
                                         s_sb[:, o:o + 2 * cols],
                                         u_ps[:, o:o + 2 * cols])
                if cf_c0 is None:
                    return a1
                a2 = apool.tile([128, 1024], BF16, tag="a2")
                for t in range(IC):
                    off = (t // 2) * 512 + (t % 2) * cols
                    nc.vector.tensor_mul(a2[:, off:off + cols],
                                         a1[:, off:off + cols],
                                         cf_bc[:, cf_c0:cf_c0 + cols])
                return a2

            def emit_down(a_t, cols, C_slot, tg, wa_t, ysb, out_ap, last_tg,
                          el=0):
                """Down-proj y^T = Wd^T-chunks @ a, copy to bf16, DMA out."""
                for grp in range(2):
                    yT = pspool.tile([128, 1024], F32, tag="y", bufs=2)
                    for j in range(4):
                        hh = grp * 4 + j
                        offy = (j // 2) * 512 + (j % 2) * cols
                        for t in range(IC):
                            offa = (t // 2) * 512 + (t % 2) * cols
                            nc.tensor.matmul(
                                yT[:, offy:offy + cols],
                                wa_t[:, 2 * W + t * H + hh * 128:
                                     2 * W + t * H + (hh + 1) * 128],
                                a_t[:, offa:offa + cols],
                                start=(t == 0), stop=(t == IC - 1))
                    if cols == C_slot:  # contiguous pair copies
                        for half in range(2):
                            hh = grp * 4 + half * 2
                            dst = ysb[:, hh * C_slot:(hh + 2) * C_slot]
                            src = yT[:, half * 512:half * 512 + 2 * cols]
                            if half == 0:
                                nc.scalar.activation(dst, src, COPY)
                            else:
                                nc.vector.tensor_copy(dst, src)
                    else:
                        for j in range(4):
                            hh = grp * 4 + j
                            offy = (j // 2) * 512 + (j % 2) * cols
                            dst = ysb[:, hh * C_slot + tg:hh * C_slot + tg + cols]
                            src = yT[:, offy:offy + cols]
                            if j % 2 == 0:
                                nc.scalar.activation(dst, src, COPY)
                            else:
                                nc.vector.tensor_copy(dst, src)
                    if last_tg and el >= ELOC - 1:
                        # tail items: ship each half as soon as its copies
                        # land (HWDGE ring; nothing left to head-block)
                        g0, g1 = grp * 4 * C_slot, (grp * 4 + 4) * C_slot
                        nc.sync.dma_start(out_ap[:, g0:g1], ysb[:, g0:g1])
                if last_tg and el < ELOC - 1:
                    nc.gpsimd.dma_start(out_ap, ysb[:, :HC * C_slot])

            for _rep in range(reps):
                pending = [None]

                def flush():
                    if pending[0] is not None:
                        pending[0]()
                        pending[0] = None

                for el in range(ELOC):
                    C = caps[el]
                    wa_t = wpool.tile([128, 3 * W], FP8, tag="wa")
                    xg_t = xpool.tile([128, HC * maxcap], FP8, tag="xg")
                    # Weights in two pieces: gate+up (sync ring) releases the
                    # gate matmuls as soon as it lands; down (scalar ring)
                    # arrives during gate/up compute. Expert 0 fans out over
                    # both rings in small pieces for the fastest first matmul.
                    if el == 0:
                        nc.scalar.dma_start(xg_t[:, :HC * C],
                                            xg_d[:, xoff[el]:xoff[el + 1]])
                        nc.sync.dma_start(wa_t[:, :W // 2],
                                          wa_d[el][:, :W // 2])
                        nc.scalar.dma_start(wa_t[:, W // 2:W],
                                            wa_d[el][:, W // 2:W])
                        nc.sync.dma_start(wa_t[:, W:2 * W],
                                          wa_d[el][:, W:2 * W])
                        nc.sync.dma_start(wa_t[:, 2 * W:],
                                          wa_d[el][:, 2 * W:])
                    else:
                        # sync ring belongs to expert slabs ALONE: the SDMA
                        # engines round-robin across busy rings, so any other
                        # traffic there starves the knife-edge weight stream.
                        nc.scalar.dma_start(xg_t[:, :HC * C],
                                            xg_d[:, xoff[el]:xoff[el + 1]])
                        nc.sync.dma_start(wa_t[:], wa_d[el][:])
                    if el == 0:  # routing coefs (host-pre-broadcast; a
                        # device partition_broadcast hotspots DMA engine 0)
                        nc.gpsimd.dma_start(cf_bc[:], cf_d[:])
                    ysb = ypool.tile([128, HC * maxcap], FP8E4, tag="ysb")
                    out_ap = ygT_d[:, xoff[el]:xoff[el + 1]]
                    for tg in range(0, C, TGMAX):
                        cols = min(TGMAX, C - tg)
                        a_t = emit_gu(xg_t, C, tg, cols, wa_t,
                                      1.0 / (WSCALE * XSCALE), yoff[el] + tg)
                        flush()
                        last = tg + cols >= C
                        pending[0] = (lambda a_t=a_t, cols=cols, C=C, tg=tg,
                                      wa_t=wa_t, ysb=ysb, out_ap=out_ap,
                                      el=el, last=last:
                                      emit_down(a_t, cols, C, tg, wa_t, ysb,
                                                out_ap, last, el))
                    if 1 <= el <= 6:
                        # shared-expert loads in six 0.5 MB chunks on the
                        # scalar ring, each held back by a marker write (WAR
                        # on was_t) so the dep-free 3 MB stream trickles into
                        # per-expert slack instead of bunching anywhere.
                        ch = W // 2
                        lo, hi = (el - 1) * ch, el * ch
                        nc.vector.tensor_copy(was_t[:, lo:lo + 1], a_t[:, 0:1])
                        nc.scalar.dma_start(was_t[:, lo:hi], was_d[:, lo:hi])
                        if el == 1:
                            nc.vector.tensor_copy(xs_t[:, 0:1], a_t[:, 0:1])
                            nc.scalar.dma_start(xs_t[:], xs_d[:])
                    if el == ELOC - 2:
                        # shared expert interleaved BEFORE the last expert so
                        # its long serial tail (silu -> mul -> down -> bf16
                        # copies -> ysT DMA) hides under el7's compute; the
                        # kernel then ends on the smallest fp8 expert.
                        a_sh = emit_gu(xs_t, TSH, 0, TSH, was_t, 1.0, None)
                        flush()
                        ysb_s = ypool.tile([128, HC * maxcap], BF16,
                                           tag="ysbs", bufs=1)
                        pending[0] = (lambda a_sh=a_sh, ysb_s=ysb_s:
                                      emit_down(a_sh, TSH, TSH, 0, was_t,
                                                ysb_s, ysT_d[:, :], True,
                                                ELOC))
                flush()

    nc.compile()
    return nc


def prepare(hidden_states, router_w, shared_gate_w, shared_up_w, shared_down_w,
            expert_gate_k, expert_up_k, expert_down_k, reps=1):
    """Host-side routing + dispatch. Returns (nc, in_maps, meta)."""
    x = np.ascontiguousarray(np.asarray(hidden_states, dtype=np.float32).reshape(T, H))
    rw = np.asarray(router_w, dtype=np.float32)
    sgw = np.asarray(shared_gate_w, dtype=np.float32)
    suw = np.asarray(shared_up_w, dtype=np.float32)
    sdw = np.asarray(shared_down_w, dtype=np.float32)

    # ---- routing on host (fp64; selection margin >> fp32 noise) ----
    logits = x.astype(np.float64) @ rw.astype(np.float64)
    aff = 1.0 / (1.0 + np.exp(-logits))
    top_idx = np.argpartition(-aff, TOPK - 1, axis=1)[:, :TOPK]        # [T,8]
    top_vals = np.take_along_axis(aff, top_idx, axis=1)
    top_w = top_vals / (top_vals.sum(axis=1, keepdims=True) + 1e-9)    # [T,8]

    flat_e = top_idx.ravel()
    flat_t = np.repeat(np.arange(T), TOPK)
    flat_w = top_w.ravel()
    order = np.argsort(flat_e, kind="stable")
    se, st, sw = flat_e[order], flat_t[order], flat_w[order]
    counts = np.bincount(flat_e, minlength=E)
    offs = np.concatenate([[0], np.cumsum(counts)])

    # count-sorted assignment: slot el gets the el-th group of 8 heaviest
    # experts (one per core) -> light slots get smaller capacities.
    perm = np.argsort(-counts, kind="stable")          # experts by load desc
    slot_expert = perm.reshape(ELOC, NCORES)           # [slot, core] -> expert
    caps = [int(max(1, counts[slot_expert[el]].max())) for el in range(ELOC)]
    xoff = np.concatenate([[0], np.cumsum([HC * c for c in caps])])
    yoff = np.concatenate([[0], np.cumsum(caps)])

    nc = _build(caps, reps=reps)

    x16 = x.astype(bfloat16)
    x8 = np.clip(x * XSCALE, -15.5, 15.5).astype(float8_e3m4)

    def q8(w):  # fp32 [.., n*128, F] weights -> scaled fp8 e3m4
        return np.clip(w * WSCALE, -15.5, 15.5).astype(float8_e3m4)

    egk8 = q8(np.asarray(expert_gate_k, dtype=np.float32))
    euk8 = q8(np.asarray(expert_up_k, dtype=np.float32))
    edk8 = q8(np.asarray(expert_down_k, dtype=np.float32))

    in_maps = []
    for c in range(NCORES):
        xg = np.zeros((128, int(xoff[-1])), float8_e3m4)
        cf = np.zeros((1, int(yoff[-1])), np.float32)  # broadcast to 128 below
        for el in range(ELOC):
            e = int(slot_expert[el, c])
            C = caps[el]
            toks = st[offs[e]:offs[e + 1]]
            ws = sw[offs[e]:offs[e + 1]]
            n = len(toks)
            xe = np.zeros((C, H), float8_e3m4)
            xe[:n] = x8[toks]
            xg[:, xoff[el]:xoff[el + 1]] = _pmajor(np.ascontiguousarray(xe.T), HC)
            # y_psum = (WSCALE*Wd)^T (silu(g) * u*WSCALE*XSCALE * cf); want
            # stored fp8 = YSCALE*coef*y_true -> cf = YSCALE*coef/(W*W*X)
            cf[0, yoff[el]:yoff[el] + n] = ws * YSCALE / (WSCALE * WSCALE * XSCALE)

        def wstack(w, nchunk):  # [ELOC, nchunk*128, F] -> [ELOC, 128, nchunk*F]
            F = w.shape[2]
            return np.ascontiguousarray(
                w.reshape(ELOC, nchunk, 128, F).transpose(0, 2, 1, 3)
                 .reshape(ELOC, 128, nchunk * F))

        eids = slot_expert[:, c]
        wa = np.concatenate([
            wstack(np.ascontiguousarray(egk8[eids]), HC),
            wstack(np.ascontiguousarray(euk8[eids]), HC),
            wstack(np.ascontiguousarray(edk8[eids]), IC),
        ], axis=2)
        was = np.concatenate([
            _pmajor(sgw.astype(bfloat16), HC),
            _pmajor(suw.astype(bfloat16), HC),
            _pmajor(sdw.astype(bfloat16), IC),
        ], axis=1)
        in_maps.append({
            "xg": xg,
            "wa": np.ascontiguousarray(wa),
            "cf": np.ascontiguousarray(np.broadcast_to(cf.astype(bfloat16), (128, cf.shape[1]))),
            "xs": _pmajor(np.ascontiguousarray(x16[TSH * c:TSH * (c + 1)].T), HC),
            "was": np.ascontiguousarray(was),
        })

    return nc, in_maps, (st, offs, slot_expert, caps, xoff, yoff)


def assemble(results, meta):
    st, offs, slot_expert, caps, xoff, yoff = meta
    out = np.zeros((T, H), np.float32)
    for c in range(NCORES):
        r = results[c]
        ysT = np.asarray(r["ysT"], np.float32)       # [128, HC*TSH]
        ys = ysT.reshape(128, HC, TSH).transpose(1, 0, 2).reshape(H, TSH)
        out[TSH * c:TSH * (c + 1)] += ys.T
        ygT = np.asarray(r["ygT"], np.float32) / YSCALE   # [128, XW]
        for el in range(ELOC):
            e = int(slot_expert[el, c])
            toks = st[offs[e]:offs[e + 1]]
            C = caps[el]
            blk = ygT[:, xoff[el]:xoff[el + 1]]
            ye = blk.reshape(128, HC, C).transpose(1, 0, 2).reshape(H, C)
            out[toks] += ye[:, :len(toks)].T
    return out.reshape(B, S, H)


def kernel(**inputs):
    global LAST_RESULT
    import os, time
    from concourse.bass_utils import run_bass_kernel_spmd
    if os.environ.get("BASS_TRACE"):
        try:
            import antenv.axon_hooks  # noqa: F401
        except ImportError:
            # trace requested but the axon NTFF hook module isn't present in
            # this container -- tracing would crash mid-run; disable it.
            os.environ["BASS_NEVER_TRACE"] = "1"
    nc, in_maps, meta = prepare(**inputs)
    last_err = None
    for attempt in range(3):
        try:
            res = run_bass_kernel_spmd(nc, in_maps, core_ids=list(range(NCORES)))
            break
        except Exception as err:  # transient device faults (e.g. NRT exec errors)
            last_err = err
            time.sleep(5 * (attempt + 1))
    else:
        raise last_err
    LAST_RESULT = res
    return assemble(res.results, meta)


# revision 26
# speedup vs baseline: 1.0734x; 1.0734x over previous
"""DeepSeek-V3.1 MoE block (B=2,S=512,H=1024,I=512,E=64,topK=8) on 8 trn2 cores.

Strategy v3 (expert-parallel, sparse dispatch, fp8 weights+activations):
  - Host: fp64 router, top-8/token, per-expert token gather with counts-sorted
    slot assignment (8 experts/core), capacity = per-slot max count.
  - Expert weights AND xg activations in fp8 e3m4 (4 mantissa bits, ~1.2% rms;
    scales 64/4 folded into the silu input scale and routing coef); routed
    output in fp8 e4m3 (scale 64 folded into coef). Shared-expert path stays
    bf16 end-to-end (it dominates output rms, its error hits ~1:1).
    Measured fro_rel 1.385e-2 vs the 2e-2 gate.
  - Device, per expert slot: one token group of C<=256 columns.
      gate/up: stationary fp8 W-chunks [128h,128i] x moving X^T [128h,C] fp8,
        accumulated over 8 h-chunks into PSUM chunks (2 banks each).
      silu(scale=1/256) on ACT; a = silu(g)*u (DVE) * coef (DVE, coef row
        pre-broadcast from host, bf16).
      down: stationary fp8 Wd-chunks [128i,128h] x moving a [128i,C] bf16 ->
        y^T [128h,C] PSUM (tokens on the free dim: no remainder-block waste),
        copied to fp8e4 and DMA'd transposed (host untransposes, /64).
  - PE software pipeline: down(e-1) issued between up(e) and gate(e+1) so
    ACT/DVE work of expert e overlaps PE down of e-1; the shared expert is
    interleaved BEFORE the last (smallest) fp8 expert so its serial bf16
    tail chain hides under that expert's compute.
  - DMA rings: sync ring carries ONLY the 1.5 MB expert slabs (SDMA engines
    round-robin across busy rings, so anything else there starves the
    knife-edge weight stream); xg/cf/was/xs ride scalar (was in six 0.5 MB
    chunks gated by marker writes so the dep-free 3 MB load cannot bunch);
    outputs ride gpsimd, tail outputs per-PSUM-group on sync.
  - DMA/core ~18.9 MB (was ~33 MB bf16): experts 12.6 fp8 + shared 3.3 bf16 +
    acts ~3 MB -> ~53 us at the ~358 GB/s HBM-per-core roofline; PE ~54-57 us
    busy (864 matmuls, avg N~140). Measured 80-84 us NEFF span (baseline 106;
    chip clock state drifts +-15% run-to-run).
"""
import os as _os, sys
try:
    import concourse  # noqa: F401  (env-provided, e.g. axon boot path)
except ImportError:
    for _p in ('/root/.axon_site/_ro/trn_rl_repo', '/opt/trn_rl_repo'):
        if _os.path.isdir(_p) and _p not in sys.path:
            sys.path.append(_p)
import numpy as np
from ml_dtypes import bfloat16, float8_e3m4, float8_e4m3

B, S, H, I, E, TOPK = 2, 512, 1024, 512, 64, 8
T = B * S
NCORES = 8
ELOC = E // NCORES
HC, IC = H // 128, I // 128
TSH = T // NCORES  # shared-expert tokens per core (128)
W = HC * I         # 4096: per-matrix partition-major width
WSCALE = 64.0      # fp8 weight scale (weights ~N(0,0.02) -> sigma ~1.28)
TGMAX = 256        # max token-group columns (PSUM chunk pair per bank)
XSCALE = 4.0       # fp8 activation scale for xg (x ~N(0,1))
YSCALE = 64.0      # fp8 output scale for ygT (folded into cf)

LAST_RESULT = None  # BassKernelResults of the most recent run (for harness)


def _pmajor(a, nchunk):
    """[nchunk*128, F] -> partition-major [128, nchunk*F] (chunk-row-major)."""
    F = a.shape[1]
    return np.ascontiguousarray(
        a.reshape(nchunk, 128, F).transpose(1, 0, 2).reshape(128, nchunk * F))


def _build(caps, reps=1):
    import concourse.bacc as bacc
    import concourse.mybir as mybir
    from concourse import tile

    F32 = mybir.dt.float32
    BF16 = mybir.dt.bfloat16
    FP8 = mybir.dt.float8e3
    FP8E4 = mybir.dt.float8e4
    SILU = mybir.ActivationFunctionType.Silu
    COPY = mybir.ActivationFunctionType.Copy

    xoff = np.concatenate([[0], np.cumsum([HC * c for c in caps])])
    yoff = np.concatenate([[0], np.cumsum(caps)])
    XW, YW = int(xoff[-1]), int(yoff[-1])
    maxcap = max(caps)

    nc = bacc.Bacc("TRN2", target_bir_lowering=False, debug=False)

    xg_d = nc.dram_tensor("xg", [128, XW], FP8, kind="ExternalInput")
    wa_d = nc.dram_tensor("wa", [ELOC, 128, 3 * W], FP8, kind="ExternalInput")
    cf_d = nc.dram_tensor("cf", [128, YW], BF16, kind="ExternalInput")
    xs_d = nc.dram_tensor("xs", [128, HC * TSH], BF16, kind="ExternalInput")
    was_d = nc.dram_tensor("was", [128, 3 * W], BF16, kind="ExternalInput")
    ygT_d = nc.dram_tensor("ygT", [128, XW], FP8E4, kind="ExternalOutput")
    ysT_d = nc.dram_tensor("ysT", [128, HC * TSH], BF16, kind="ExternalOutput")

    with tile.TileContext(nc) as tc:
        with (
            tc.tile_pool(name="const", bufs=1) as cpool,
            tc.tile_pool(name="wp", bufs=3) as wpool,
            tc.tile_pool(name="xp", bufs=3) as xpool,
            tc.tile_pool(name="ap", bufs=2) as apool,
            tc.tile_pool(name="yp", bufs=2) as ypool,
            tc.tile_pool(name="ps", bufs=1, space="PSUM") as pspool,
        ):
            cf_bc = cpool.tile([128, YW], BF16)
            was_t = cpool.tile([128, 3 * W], BF16)
            xs_t = cpool.tile([128, HC * TSH], BF16)

            def emit_gu(xg_t, C_slot, tg, cols, wa_t, silu_scale, cf_c0):
                """Gate+up+silu+mul for one token group; returns the a tile."""
                g_ps = pspool.tile([128, 1024], F32, tag="g", bufs=1)
                u_ps = pspool.tile([128, 1024], F32, tag="u", bufs=1)
                for base, ps in ((0, g_ps), (W, u_ps)):
                    for t in range(IC):
                        off = (t // 2) * 512 + (t % 2) * cols
                        for h in range(HC):
                            nc.tensor.matmul(
                                ps[:, off:off + cols],
                                wa_t[:, base + h * I + t * 128:
                                     base + h * I + (t + 1) * 128],
                                xg_t[:, h * C_slot + tg:h * C_slot + tg + cols],
                                start=(h == 0), stop=(h == HC - 1))
                s_sb = apool.tile([128, 1024], F32, tag="s")
                a1 = apool.tile([128, 1024], BF16, tag="a1")
                for o in (0, 512):
                    nc.scalar.activation(s_sb[:, o:o + 2 * cols],
                                         g_ps[:, o:o + 2 * cols], SILU,
                                         scale=silu_scale)
                    nc.vector.tensor_mul(a1[:, o:o + 2 * cols],
                                         s_sb[:, o:o + 2 * cols],
                                         u_ps[:, o:o + 2 * cols])
                if cf_c0 is None:
                    return a1
                a2 = apool.tile([128, 1024], BF16, tag="a2")
                for t in range(IC):
                    off = (t // 2) * 512 + (t % 2) * cols
                    nc.vector.tensor_mul(a2[:, off:off + cols],
                                         a1[:, off:off + cols],
                                         cf_bc[:, cf_c0:cf_c0 + cols])
                return a2

            def emit_down(a_t, cols, C_slot, tg, wa_t, ysb, out_ap, last_tg,
                          el=0):
                """Down-proj y^T = Wd^T-chunks @ a, copy to bf16, DMA out."""
                for grp in range(2):
                    yT = pspool.tile([128, 1024], F32, tag="y", bufs=2)
                    for j in range(4):
                        hh = grp * 4 + j
                        offy = (j // 2) * 512 + (j % 2) * cols
                        for t in range(IC):
                            offa = (t // 2) * 512 + (t % 2) * cols
                            nc.tensor.matmul(
                                yT[:, offy:offy + cols],
                                wa_t[:, 2 * W + t * H + hh * 128:
                                     2 * W + t * H + (hh + 1) * 128],
                                a_t[:, offa:offa + cols],
                                start=(t == 0), stop=(t == IC - 1))
                    if cols == C_slot:  # contiguous pair copies
                        for half in range(2):
                            hh = grp * 4 + half * 2
                            dst = ysb[:, hh * C_slot:(hh + 2) * C_slot]
                            src = yT[:, half * 512:half * 512 + 2 * cols]
                            if half == 0:
                                nc.scalar.activation(dst, src, COPY)
                            else:
                                nc.vector.tensor_copy(dst, src)
                    else:
                        for j in range(4):
                            hh = grp * 4 + j
                            offy = (j // 2) * 512 + (j % 2) * cols
                            dst = ysb[:, hh * C_slot + tg:hh * C_slot + tg + cols]
                            src = yT[:, offy:offy + cols]
                            if j % 2 == 0:
                                nc.scalar.activation(dst, src, COPY)
                            else:
                                nc.vector.tensor_copy(dst, src)
                    if last_tg and el >= ELOC - 1:
                        # tail items: ship each half as soon as its copies
                        # land (HWDGE ring; nothing left to head-block)
                        g0, g1 = grp * 4 * C_slot, (grp * 4 + 4) * C_slot
                        nc.sync.dma_start(out_ap[:, g0:g1], ysb[:, g0:g1])
                if last_tg and el < ELOC - 1:
                    nc.gpsimd.dma_start(out_ap, ysb[:, :HC * C_slot])

            for _rep in range(reps):
                pending = [None]

                def flush():
                    if pending[0] is not None:
                        pending[0]()
                        pending[0] = None

                for el in range(ELOC):
                    C = caps[el]
                    wa_t = wpool.tile([128, 3 * W], FP8, tag="wa")
                    xg_t = xpool.tile([128, HC * maxcap], FP8, tag="xg")
                    # Weights in two pieces: gate+up (sync ring) releases the
                    # gate matmuls as soon as it lands; down (scalar ring)
                    # arrives during gate/up compute. Expert 0 fans out over
                    # both rings in small pieces for the fastest first matmul.
                    if el == 0:
                        nc.scalar.dma_start(xg_t[:, :HC * C],
                                            xg_d[:, xoff[el]:xoff[el + 1]])
                        nc.sync.dma_start(wa_t[:, :W // 2],
                                          wa_d[el][:, :W // 2])
                        nc.scalar.dma_start(wa_t[:, W // 2:W],
                                            wa_d[el][:, W // 2:W])
                        nc.sync.dma_start(wa_t[:, W:2 * W],
                                          wa_d[el][:, W:2 * W])
                        nc.sync.dma_start(wa_t[:, 2 * W:],
                                          wa_d[el][:, 2 * W:])
                    else:
                        # sync ring belongs to expert slabs ALONE: the SDMA
                        # engines round-robin across busy rings, so any other
                        # traffic there starves the knife-edge weight stream.
                        nc.scalar.dma_start(xg_t[:, :HC * C],
                                            xg_d[:, xoff[el]:xoff[el + 1]])
                        nc.sync.dma_start(wa_t[:], wa_d[el][:])
                    if el == 0:  # routing coefs (host-pre-broadcast); on
                        # scalar AFTER el0's gate piece: a t=0 load on the
                        # idle gpsimd ring would cut startup bandwidth by 1/3
                        nc.scalar.dma_start(cf_bc[:], cf_d[:])
                    ysb = ypool.tile([128, HC * maxcap], FP8E4, tag="ysb")
                    out_ap = ygT_d[:, xoff[el]:xoff[el + 1]]
                    for tg in range(0, C, TGMAX):
                        cols = min(TGMAX, C - tg)
                        a_t = emit_gu(xg_t, C, tg, cols, wa_t,
                                      1.0 / (WSCALE * XSCALE), yoff[el] + tg)
                        flush()
                        last = tg + cols >= C
                        pending[0] = (lambda a_t=a_t, cols=cols, C=C, tg=tg,
                                      wa_t=wa_t, ysb=ysb, out_ap=out_ap,
                                      el=el, last=last:
                                      emit_down(a_t, cols, C, tg, wa_t, ysb,
                                                out_ap, last, el))
                    if 1 <= el <= 6:
                        # shared-expert loads in six 0.5 MB chunks on the
                        # scalar ring, each held back by a marker write (WAR
                        # on was_t) so the dep-free 3 MB stream trickles into
                        # per-expert slack instead of bunching anywhere.
                        ch = W // 2
                        lo, hi = (el - 1) * ch, el * ch
                        nc.vector.tensor_copy(was_t[:, lo:lo + 1], a_t[:, 0:1])
                        nc.scalar.dma_start(was_t[:, lo:hi], was_d[:, lo:hi])
                        if el == 1:
                            nc.vector.tensor_copy(xs_t[:, 0:1], a_t[:, 0:1])
                            nc.scalar.dma_start(xs_t[:], xs_d[:])
                    if el == ELOC - 2:
                        # shared expert interleaved BEFORE the last expert so
                        # its long serial tail (silu -> mul -> down -> bf16
                        # copies -> ysT DMA) hides under el7's compute; the
                        # kernel then ends on the smallest fp8 expert.
                        a_sh = emit_gu(xs_t, TSH, 0, TSH, was_t, 1.0, None)
                        flush()
                        ysb_s = ypool.tile([128, HC * maxcap], BF16,
                                           tag="ysbs", bufs=1)
                        pending[0] = (lambda a_sh=a_sh, ysb_s=ysb_s:
                                      emit_down(a_sh, TSH, TSH, 0, was_t,
                                                ysb_s, ysT_d[:, :], True,
                                                ELOC))
                flush()

    nc.compile()
    return nc


def prepare(hidden_states, router_w, shared_gate_w, shared_up_w, shared_down_w,
            expert_gate_k, expert_up_k, expert_down_k, reps=1):
    """Host-side routing + dispatch. Returns (nc, in_maps, meta)."""
    x = np.ascontiguousarray(np.asarray(hidden_states, dtype=np.float32).reshape(T, H))
    rw = np.asarray(router_w, dtype=np.float32)
    sgw = np.asarray(shared_gate_w, dtype=np.float32)
    suw = np.asarray(shared_up_w, dtype=np.float32)
    sdw = np.asarray(shared_down_w, dtype=np.float32)

    # ---- routing on host (fp64; selection margin >> fp32 noise) ----
    logits = x.astype(np.float64) @ rw.astype(np.float64)
    aff = 1.0 / (1.0 + np.exp(-logits))
    top_idx = np.argpartition(-aff, TOPK - 1, axis=1)[:, :TOPK]        # [T,8]
    top_vals = np.take_along_axis(aff, top_idx, axis=1)
    top_w = top_vals / (top_vals.sum(axis=1, keepdims=True) + 1e-9)    # [T,8]

    flat_e = top_idx.ravel()
    flat_t = np.repeat(np.arange(T), TOPK)
    flat_w = top_w.ravel()
    order = np.argsort(flat_e, kind="stable")
    se, st, sw = flat_e[order], flat_t[order], flat_w[order]
    counts = np.bincount(flat_e, minlength=E)
    offs = np.concatenate([[0], np.cumsum(counts)])

    # count-sorted assignment: slot el gets the el-th group of 8 heaviest
    # experts (one per core) -> light slots get smaller capacities.
    perm = np.argsort(-counts, kind="stable")          # experts by load desc
    slot_expert = perm.reshape(ELOC, NCORES)           # [slot, core] -> expert
    caps = [int(max(1, counts[slot_expert[el]].max())) for el in range(ELOC)]
    xoff = np.concatenate([[0], np.cumsum([HC * c for c in caps])])
    yoff = np.concatenate([[0], np.cumsum(caps)])

    nc = _build(caps, reps=reps)

    x16 = x.astype(bfloat16)
    x8 = np.clip(x * XSCALE, -15.5, 15.5).astype(float8_e3m4)

    def q8(w):  # fp32 [.., n*128, F] weights -> scaled fp8 e3m4
        return np.clip(w * WSCALE, -15.5, 15.5).astype(float8_e3m4)

    egk8 = q8(np.asarray(expert_gate_k, dtype=np.float32))
    euk8 = q8(np.asarray(expert_up_k, dtype=np.float32))
    edk8 = q8(np.asarray(expert_down_k, dtype=np.float32))

    in_maps = []
    for c in range(NCORES):
        xg = np.zeros((128, int(xoff[-1])), float8_e3m4)
        cf = np.zeros((1, int(yoff[-1])), np.float32)  # broadcast to 128 below
        for el in range(ELOC):
            e = int(slot_expert[el, c])
            C = caps[el]
            toks = st[offs[e]:offs[e + 1]]
            ws = sw[offs[e]:offs[e + 1]]
            n = len(toks)
            xe = np.zeros((C, H), float8_e3m4)
            xe[:n] = x8[toks]
            xg[:, xoff[el]:xoff[el + 1]] = _pmajor(np.ascontiguousarray(xe.T), HC)
            # y_psum = (WSCALE*Wd)^T (silu(g) * u*WSCALE*XSCALE * cf); want
            # stored fp8 = YSCALE*coef*y_true -> cf = YSCALE*coef/(W*W*X)
            cf[0, yoff[el]:yoff[el] + n] = ws * YSCALE / (WSCALE * WSCALE * XSCALE)

        def wstack(w, nchunk):  # [ELOC, nchunk*128, F] -> [ELOC, 128, nchunk*F]
            F = w.shape[2]
            return np.ascontiguousarray(
                w.reshape(ELOC, nchunk, 128, F).transpose(0, 2, 1, 3)
                 .reshape(ELOC, 128, nchunk * F))

        eids = slot_expert[:, c]
        wa = np.concatenate([
            wstack(np.ascontiguousarray(egk8[eids]), HC),
            wstack(np.ascontiguousarray(euk8[eids]), HC),
            wstack(np.ascontiguousarray(edk8[eids]), IC),
        ], axis=2)
        was = np.concatenate([
            _pmajor(sgw.astype(bfloat16), HC),
            _pmajor(suw.astype(bfloat16), HC),
            _pmajor(sdw.astype(bfloat16), IC),
        ], axis=1)
        in_maps.append({
            "xg": xg,
            "wa": np.ascontiguousarray(wa),
            "cf": np.ascontiguousarray(np.broadcast_to(cf.astype(bfloat16), (128, cf.shape[1]))),
            "xs": _pmajor(np.ascontiguousarray(x16[TSH * c:TSH * (c + 1)].T), HC),
            "was": np.ascontiguousarray(was),
        })

    return nc, in_maps, (st, offs, slot_expert, caps, xoff, yoff)


def assemble(results, meta):
    st, offs, slot_expert, caps, xoff, yoff = meta
    out = np.zeros((T, H), np.float32)
    for c in range(NCORES):
        r = results[c]
        ysT = np.asarray(r["ysT"], np.float32)       # [128, HC*TSH]
        ys = ysT.reshape(128, HC, TSH).transpose(1, 0, 2).reshape(H, TSH)
        out[TSH * c:TSH * (c + 1)] += ys.T
        ygT = np.asarray(r["ygT"], np.float32) / YSCALE   # [128, XW]
        for el in range(ELOC):
            e = int(slot_expert[el, c])
            toks = st[offs[e]:offs[e + 1]]
            C = caps[el]
            blk = ygT[:, xoff[el]:xoff[el + 1]]
            ye = blk.reshape(128, HC, C).transpose(1, 0, 2).reshape(H, C)
            out[toks] += ye[:, :len(toks)].T
    return out.reshape(B, S, H)


def kernel(**inputs):
    global LAST_RESULT
    import os, time
    from concourse.bass_utils import run_bass_kernel_spmd
    if os.environ.get("BASS_TRACE"):
        try:
            import antenv.axon_hooks  # noqa: F401
        except ImportError:
            # trace requested but the axon NTFF hook module isn't present in
            # this container -- tracing would crash mid-run; disable it.
            os.environ["BASS_NEVER_TRACE"] = "1"
    nc, in_maps, meta = prepare(**inputs)
    last_err = None
    for attempt in range(3):
        try:
            res = run_bass_kernel_spmd(nc, in_maps, core_ids=list(range(NCORES)))
            break
        except Exception as err:  # transient device faults (e.g. NRT exec errors)
            last_err = err
            time.sleep(5 * (attempt + 1))
    else:
        raise last_err
    LAST_RESULT = res
    return assemble(res.results, meta)
